# revision 52
# baseline (speedup 1.0000x reference)
"""Deformable conv block (offset conv -> bilinear sample -> conv -> BN -> ReLU)
on 8 Trainium2 NeuronCores, data-parallel over batch.

Self-contained: hardcodes all shapes. kernel(**inputs) takes full inputs,
shards batch across 8 cores, runs one Bass/Tile SPMD program, returns the
full [8, 64, 96, 96] float32 output.

v4 (fused chunk pipeline):
  - single main loop over 6 chunks of 16 image rows (1536 px, 12 blocks);
    per chunk: offset conv (PE) -> pixel/tap idx + bilinear weights (DVE,
    all 9 taps in one set of wide ops) -> 9 dma_gathers (GpSimd SWDGE)
    -> bilinear combine (DVE bf16) -> pair transpose (PE) -> conv matmuls
    (PE, fp32 PSUM accumulate) -> chunk BN partial sums (Scalar)
  - gather table rows hold all 4 bilinear corners channel-interleaved in
    bf16 (512B rows) -> one 512B descriptor per (pixel, tap)
  - GpSimd runs ONLY gathers; with deep tile rings they issue back-to-back
    (~703us of SWDGE descriptor-gen is the kernel floor)
  - pair transposes on the PE (TensorMatrix) instead of the XBAR: keeps
    the SWDGE descriptor rings gather-only (no cross FIFO stalls)
  - two-phase BN stats AllReduce: chunks 0-4 reduced mid-loop (hides the
    ~23us cross-core arrival skew under chunk 5's gathers), chunk 5
    reduced at the end (cores already synced -> short wait); then
    scale/bias fold, ReLU, un-permute out
"""
import os
from contextlib import ExitStack

import numpy as np
import ml_dtypes

import concourse.bass as bass
import concourse.tile as tile
from concourse import bacc, mybir, bass_utils

dt = mybir.dt
AOT = mybir.AluOpType
AFT = mybir.ActivationFunctionType

# problem shapes
B, C, H, W, K = 8, 64, 96, 96, 3
HW = H * W                # 9216
K2 = K * K                # 9
NCORES = 8
EPS = 1e-5

# padded sample-grid geometry: padded coord = image coord + PADM
PADM = 3                  # margin for floor(py) in [0, 100]
PW = W + 2 * PADM + 1     # 103 padded grid width
PR = H + 2 * PADM + 1     # 103 padded grid rows
NQ = PW * PR              # 10609 rows in the 4-corner gather table
QCLAMP = float(W + 2 * PADM - 2)  # 100: floor clamp ceiling

NB = HW // 128            # 72 pixel-major block columns
CHUNK = 12                # block columns per chunk (16 rows, 1536 px)
NCH = NB // CHUNK         # 6 chunks
NPIX = CHUNK * 128        # 1536 pixels per chunk (single_packet gather max)
NROW = 16                 # image rows per chunk
KB = K2 * CHUNK           # 108 (tap, block) pairs per chunk
MAGIC = 8388608.0         # 2^23

_CACHE = {}


def _build(nc, ncores=NCORES, use_collective=True):
    STOP = os.environ.get("KSTOP", "full")
    xpad = nc.dram_tensor("xpad", [C, 98 * 98], dt.bfloat16, kind="ExternalInput").ap()
    x2pad = nc.dram_tensor("x2pad", [NQ, 256], dt.bfloat16, kind="ExternalInput").ap()
    woffT = nc.dram_tensor("woffT", [C, K2 * 18], dt.bfloat16, kind="ExternalInput").ap()
    wdefT = nc.dram_tensor("wdefT", [128, K2 * C], dt.bfloat16, kind="ExternalInput").ap()
    bpy = nc.dram_tensor("bpy", [128, K2 * NB], dt.float32, kind="ExternalInput").ap()
    bpx = nc.dram_tensor("bpx", [128, K2 * NB], dt.float32, kind="ExternalInput").ap()
    ident = nc.dram_tensor("ident", [128, 128], dt.float32, kind="ExternalInput").ap()
    bnc = nc.dram_tensor("bnc", [C, 2], dt.float32, kind="ExternalInput").ap()
    out_d = nc.dram_tensor("out", [C, HW], dt.float32, kind="ExternalOutput").ap()

    with tile.TileContext(nc) as tc:
        with ExitStack() as ctx:
            cpool = ctx.enter_context(tc.tile_pool(name="const", bufs=1))
            ppool = ctx.enter_context(tc.tile_pool(name="persist", bufs=1))
            spool = ctx.enter_context(tc.tile_pool(name="small", bufs=3))
            gpool = ctx.enter_context(tc.tile_pool(name="gather", bufs=4))
            tpool = ctx.enter_context(tc.tile_pool(name="tmul", bufs=3))
            wpool = ctx.enter_context(tc.tile_pool(name="work", bufs=3))
            dpool = ctx.enter_context(tc.tile_pool(name="dram", bufs=1, space="DRAM"))
            ps_m = ctx.enter_context(tc.tile_pool(name="ps_m", bufs=2, space="PSUM"))
            ps_o = ctx.enter_context(tc.tile_pool(name="ps_o", bufs=1, space="PSUM"))
            ps_t = ctx.enter_context(tc.tile_pool(name="ps_t", bufs=2, space="PSUM"))

            # ---- load constants ----
            woffT_s = cpool.tile([C, K2 * 18], dt.bfloat16)
            nc.sync.dma_start(woffT_s[:], woffT)
            wdefT_s = cpool.tile([128, K2 * C], dt.bfloat16)
            nc.sync.dma_start(wdefT_s[:], wdefT)
            bpy_s = cpool.tile([128, K2 * NB], dt.float32)
            nc.sync.dma_start(bpy_s[:], bpy)
            bpx_s = cpool.tile([128, K2 * NB], dt.float32)
            nc.sync.dma_start(bpx_s[:], bpx)
            id_s = cpool.tile([128, 128], dt.float32)
            nc.sync.dma_start(id_s[:], ident)
            id_b = cpool.tile([128, 128], dt.bfloat16)
            nc.vector.tensor_copy(id_b[:], id_s[:])
            bnc_s = cpool.tile([C, 2], dt.float32)
            nc.sync.dma_start(bnc_s[:], bnc)

            # persistent state
            conv_s = ppool.tile([C, HW], dt.float32)
            SEGS = [(12 * i, 12, i) for i in range(6)]
            NSEG = len(SEGS)
            sums = ppool.tile([C, 2 * NSEG], dt.float32)
            sqs = ppool.tile([C, NSEG], dt.float32)
            bpyv = bpy_s[:].rearrange("p (k b) -> p k b", k=K2)
            bpxv = bpx_s[:].rearrange("p (k b) -> p k b", k=K2)
            x2win = bass.AP(x2pad.tensor, 0, [[256, NQ], [1, 256]])
            xpv = xpad.rearrange("c (h w) -> c h w", w=98)

            NIC = NPIX // 16          # idx cols per (chunk, tap): 96

            # ====== fused main loop over block segments (b0, nb) ===========
            # software-pipelined: prep(seg+1) is issued before body(seg) so
            # the in-order PE/DVE queues do next-segment prep under this
            # segment's gather-paced body.
            def prep(b0, nb):
                kb = K2 * nb
                # ---- 1. offset conv for this segment's rows ----------
                # offT[p, 18*b + j] = off[j, 128*(b0+b) + p], b local 0..nb-1
                offT_c = spool.tile([128, CHUNK * 18], dt.float32, tag="offT",
                                    bufs=2)
                for s in range(nb // 3):  # 4-row sub-chunks = 3 blocks
                    r0 = 4 * b0 // 3 + 4 * s
                    xpc = wpool.tile([C, 6 * 98], dt.bfloat16, tag="xpc")
                    nc.sync.dma_start(
                        xpc[:].rearrange("c (h w) -> c h w", w=98),
                        xpv[:, r0: r0 + 6, :])
                    xv = xpc[:].rearrange("c (h w) -> c h w", w=98)
                    po = ps_m.tile([18, 384], dt.float32, tag="ps_misc")
                    for k in range(K2):
                        ky, kx = k // K, k % K
                        rhs = xv[:, ky: ky + 4, kx: kx + 96]
                        nc.tensor.matmul(po[:], woffT_s[:, 18 * k: 18 * k + 18],
                                         rhs, start=(k == 0), stop=(k == K2 - 1))
                    offc = wpool.tile([18, 384], dt.float32, tag="offc")
                    nc.scalar.copy(offc[:], po[:])
                    for cb in range(3):
                        pt = ps_m.tile([128, 18], dt.float32, tag="ps_misc")
                        nc.tensor.transpose(pt[:], offc[:, 128 * cb: 128 * cb + 128],
                                            id_s[0:18, 0:18])
                        c = 3 * s + cb
                        nc.vector.tensor_copy(offT_c[:, 18 * c: 18 * c + 18], pt[:])
                offv = offT_c[:, 0:nb * 18].rearrange("p (b j) -> p b j", j=18)

                # ---- 2. idx + bilinear weights, all 9 taps in wide ops ----
                # [128, (k, b)] layout, k-major
                def st(tag, mult=1):
                    t = spool.tile([128, KB * mult], dt.float32, tag=tag,
                                   name=tag)
                    return t[:, 0:kb * mult]
                py = st("py")
                pyv = py.rearrange("p (k b) -> p k b", k=K2)
                nc.vector.tensor_tensor(
                    pyv, offv[:, :, 0::2].rearrange("p b k -> p k b"),
                    bpyv[:, :, b0: b0 + nb], AOT.add)
                px = st("px")
                pxv = px.rearrange("p (k b) -> p k b", k=K2)
                nc.vector.tensor_tensor(
                    pxv, offv[:, :, 1::2].rearrange("p b k -> p k b"),
                    bpxv[:, :, b0: b0 + nb], AOT.add)
                ry = st("ry")
                nc.vector.tensor_scalar(ry, py, MAGIC - 0.5, None, AOT.add)
                fy = st("fy")
                nc.vector.tensor_scalar(fy, ry, MAGIC, None, AOT.subtract)
                rx = st("rx")
                nc.vector.tensor_scalar(rx, px, MAGIC - 0.5, None, AOT.add)
                fx = st("fx")
                nc.vector.tensor_scalar(fx, rx, MAGIC, None, AOT.subtract)
                ly = st("ly")
                nc.vector.tensor_tensor(ly, py, fy, AOT.subtract)
                lx = st("lx")
                nc.vector.tensor_tensor(lx, px, fx, AOT.subtract)
                wy0 = st("wy0")
                nc.vector.tensor_scalar(wy0, ly, -1.0, 1.0, AOT.mult, AOT.add)
                wx0 = st("wx0")
                nc.vector.tensor_scalar(wx0, lx, -1.0, 1.0, AOT.mult, AOT.add)
                w4f = st("w4f", 4)
                w4fv = w4f.rearrange("p (kb j) -> p kb j", j=4)
                nc.vector.tensor_tensor(w4fv[:, :, 0], wy0, wx0, AOT.mult)
                nc.vector.tensor_tensor(w4fv[:, :, 1], ly, wx0, AOT.mult)
                nc.vector.tensor_tensor(w4fv[:, :, 2], wy0, lx, AOT.mult)
                nc.vector.tensor_tensor(w4fv[:, :, 3], ly, lx, AOT.mult)
                w4b = spool.tile([128, KB * 4], dt.bfloat16, tag="w4b", bufs=2)
                nc.vector.tensor_copy(w4b[:, 0:kb * 4], w4f)
                w4u = w4b[:, 0:kb * 4].rearrange("p (k b u j) -> p k b u j",
                                                 k=K2, u=1, j=4)
                # gather row index q = qy * PW + qx (exact small integers)
                qy = st("qy")
                nc.vector.tensor_scalar(qy, fy, 0.0, QCLAMP, AOT.max, AOT.min)
                qx = st("qx")
                nc.vector.tensor_scalar(qx, fx, 0.0, QCLAMP, AOT.max, AOT.min)
                qf = st("qf")
                nc.vector.scalar_tensor_tensor(qf, qy, float(PW), qx,
                                               AOT.mult, AOT.add)
                # 16-row wrap via PE double-transpose: T1 then 8x T2;
                # idx_ch[p, (k, b, r)] int16
                t1p = ps_m.tile([KB, 128], dt.float32, tag="ps_misc",
                                name="t1p")
                nc.tensor.transpose(t1p[0:kb, :], qf, id_s[:])
                t1s = spool.tile([KB, 128], dt.float32, tag="t1s")
                nc.vector.tensor_copy(t1s[0:kb, :], t1p[0:kb, :])
                idx_ch = spool.tile([128, K2 * NIC], dt.int16, tag="idx", bufs=3)
                stv = idx_ch[0:16, 0:kb * 8].rearrange("p (k b r) -> p k b r",
                                                       k=K2, r=8)
                for r in range(8):
                    t2p = ps_m.tile([16, KB], dt.float32, tag="ps_misc",
                                    name="t2p")
                    nc.tensor.transpose(t2p[:, 0:kb], t1s[0:kb, 16 * r: 16 * r + 16],
                                        id_s[0:kb, 0:kb])
                    nc.any.tensor_copy(stv[:, :, :, r],
                                       t2p[:, 0:kb].rearrange("p (k b) -> p k b",
                                                              k=K2))
                # replicate idx rows 0..15 to all 8 16-row groups
                nic = 8 * nb
                nc.sync.dma_start(idx_ch[16:32, 0:K2 * nic], idx_ch[0:16, 0:K2 * nic])
                nc.sync.dma_start(idx_ch[32:64, 0:K2 * nic], idx_ch[0:32, 0:K2 * nic])
                nc.sync.dma_start(idx_ch[64:128, 0:K2 * nic], idx_ch[0:64, 0:K2 * nic])
                return w4u, idx_ch

            def body(b0, nb, slot, w4u, idx_ch, after_gather0=None):
                # ---- 3. per tap: gather -> combine -> transpose -> conv ----
                npix = 128 * nb
                nic = 8 * nb
                half = npix // 2
                pieces = [(0, 512), (512, half)] if half > 512 else [(0, half)]
                po_e = ps_o.tile([C, NPIX // 2], dt.float32, tag="ps_oute")
                po_o = ps_o.tile([C, NPIX // 2], dt.float32, tag="ps_outo")
                for k in range(K2):
                    g_t = gpool.tile([128, CHUNK * 256], dt.bfloat16, tag="g")
                    nc.gpsimd.dma_gather(
                        out_ap=g_t[:, 0:nb * 256].rearrange("p (b e) -> p b e",
                                                            e=256),
                        in_ap=x2win,
                        idxs_ap=idx_ch[:, nic * k: nic * k + nic],
                        num_idxs=npix,
                        num_idxs_reg=npix,
                        elem_size=256,
                        elem_step=256,
                        queue_num=1,
                        single_packet=False,
                    )
                    if k == 0 and after_gather0 is not None:
                        after_gather0()
                    if STOP == "g1":
                        nc.sync.dma_start(
                            out_d[0:64, 0:nb * 128].bitcast(dt.bfloat16)[:, 0:nb * 256],
                            g_t[0:64, 0:nb * 256])
                        return
                    # bilinear combine: t = g * w4 (broadcast over channels),
                    # then reduce over the 4 corners (innermost)
                    gv4 = g_t[:, 0:nb * 256].rearrange("p (b c j) -> p b c j",
                                                       c=C, j=4)
                    wj = w4u[:, k]
                    a1, a2 = bass.broadcast_tensor_aps(gv4, wj)
                    t_t = tpool.tile([128, CHUNK * 256], dt.bfloat16, tag="t")
                    nc.vector.tensor_tensor(
                        t_t[:, 0:nb * 256].rearrange("p (b c j) -> p b c j",
                                                     c=C, j=4), a1, a2,
                        AOT.mult)
                    # 4-corner sum as two adds: pairs (2x-mode) then final
                    tv = t_t[:, 0:nb * 256].rearrange("p (bc j) -> p bc j", j=4)
                    s2 = wpool.tile([128, CHUNK * C * 2], dt.bfloat16, tag="s2",
                                    bufs=6)
                    s2v = s2[:, 0:nb * 128].rearrange("p (bc j) -> p bc j", j=2)
                    nc.vector.tensor_tensor(s2v, tv[:, :, 0:2], tv[:, :, 2:4],
                                            AOT.add)
                    s_t = wpool.tile([128, CHUNK * C], dt.bfloat16, tag="s",
                                     bufs=6)
                    nc.vector.tensor_tensor(s_t[:, 0:nb * C], s2v[:, :, 0],
                                            s2v[:, :, 1], AOT.add)
                    # pair transpose -> channel-major sampled (PE transposes)
                    samp = wpool.tile([128, CHUNK * C], dt.bfloat16, tag="samp",
                                      bufs=6)
                    tp_p = ps_t.tile([128, CHUNK * C], dt.bfloat16, tag="ps_tr")
                    for j in range(nb * C // 128):
                        nc.tensor.transpose(tp_p[:, 128 * j: 128 * j + 128],
                                            s_t[:, 128 * j: 128 * j + 128],
                                            id_b[:])
                    nc.scalar.copy(samp[:, 0:nb * C], tp_p[:, 0:nb * C])
                    # conv matmuls: accumulate over taps, parity-major out
                    # cols; <=512-col pieces keep each matmul inside PSUM banks
                    st, sp = (k == 0), (k == K2 - 1)
                    lhe = wdefT_s[0:64, C * k: C * k + C]
                    lho = wdefT_s[64:128, C * k: C * k + C]
                    for (a, b) in pieces:
                        nc.tensor.matmul(po_e[:, a:b], lhe, samp[0:64, a:b],
                                         start=st, stop=sp)
                        nc.tensor.matmul(po_o[:, a:b], lho, samp[64:128, a:b],
                                         start=st, stop=sp)
                # copy conv chunk to SBUF + per-chunk sum / sumsq
                cview = conv_s[:, 128 * b0: 128 * b0 + npix]
                nc.scalar.activation(cview[:, 0:half], po_e[:, 0:half], AFT.Copy,
                                     accum_out=sums[:, 2 * slot: 2 * slot + 1])
                nc.scalar.activation(cview[:, half:npix], po_o[:, 0:half],
                                     AFT.Copy,
                                     accum_out=sums[:, 2 * slot + 1: 2 * slot + 2])
                scr = wpool.tile([C, NPIX], dt.float32, tag="scr", bufs=1)
                nc.scalar.activation(scr[:, 0:npix], cview, AFT.Square,
                                     accum_out=sqs[:, slot: slot + 1])
                return None

            # two-phase BN stats allreduce: chunks 0..4 mid-loop (triggered
            # between chunk 5's gathers so the GpSimd queue never stalls),
            # chunk 5 at the end (cores already synced -> short wait)
            st2a = ppool.tile([C, 2], dt.float32)
            st2b = ppool.tile([C, 2], dt.float32)
            bi_a = dpool.tile([C, 2], dt.float32)
            bo_a = dpool.tile([C, 2], dt.float32)
            bi_b = dpool.tile([C, 2], dt.float32)
            bo_b = dpool.tile([C, 2], dt.float32)

            def coll_a():
                nc.vector.tensor_reduce(st2a[:, 0:1], sums[:, 0:2 * (NSEG - 1)],
                                        mybir.AxisListType.X, AOT.add)
                nc.vector.tensor_reduce(st2a[:, 1:2], sqs[:, 0:NSEG - 1],
                                        mybir.AxisListType.X, AOT.add)
                nc.sync.dma_start(bi_a[:], st2a[:])
                if use_collective:
                    nc.gpsimd.collective_compute(
                        "AllReduce", AOT.add,
                        replica_groups=[list(range(ncores))],
                        ins=[bi_a.opt()], outs=[bo_a.opt()])
                else:
                    nc.sync.dma_start(bo_a[:], bi_a[:])

            cur = prep(*SEGS[0][0:2])
            if STOP == "idx":
                nc.sync.dma_start(
                    out_d[0:64, 0:K2 * NIC].bitcast(dt.int16)[:, 0:K2 * NIC],
                    cur[1][0:64, :])
                return
            for i, (b0, nb, slot) in enumerate(SEGS):
                nxt = prep(*SEGS[i + 1][0:2]) if i + 1 < NSEG else None
                hook = coll_a if i == NSEG - 1 else None
                body(b0, nb, slot, *cur, after_gather0=hook)
                if STOP == "g1":
                    return
                cur = nxt

            if STOP == "loop":
                nc.sync.dma_start(out_d[:], conv_s[:])
                return

            # last-segment stats + second (short) allreduce
            lc = NSEG - 1
            nc.vector.tensor_reduce(st2b[:, 0:1], sums[:, 2 * lc: 2 * lc + 2],
                                    mybir.AxisListType.X, AOT.add)
            nc.vector.tensor_copy(st2b[:, 1:2], sqs[:, lc: lc + 1])
            nc.sync.dma_start(bi_b[:], st2b[:])
            if use_collective:
                nc.gpsimd.collective_compute(
                    "AllReduce", AOT.add,
                    replica_groups=[list(range(ncores))],
                    ins=[bi_b.opt()], outs=[bo_b.opt()])
            else:
                nc.sync.dma_start(bo_b[:], bi_b[:])
            ast_a = ppool.tile([C, 2], dt.float32)
            nc.sync.dma_start(ast_a[:], bo_a[:])
            ast_b = ppool.tile([C, 2], dt.float32)
            nc.sync.dma_start(ast_b[:], bo_b[:])
            ast = ppool.tile([C, 2], dt.float32)
            nc.vector.tensor_tensor(ast[:], ast_a[:], ast_b[:], AOT.add)

            inv_n = 1.0 / float(ncores * HW)
            mean = ppool.tile([C, 1], dt.float32)
            nc.vector.tensor_scalar(mean[:], ast[:, 0:1], inv_n, None, AOT.mult)
            msq = ppool.tile([C, 1], dt.float32)
            nc.vector.tensor_scalar(msq[:], ast[:, 1:2], inv_n, None, AOT.mult)
            m2 = ppool.tile([C, 1], dt.float32)
            nc.vector.tensor_tensor(m2[:], mean[:], mean[:], AOT.mult)
            var = ppool.tile([C, 1], dt.float32)
            nc.vector.tensor_tensor(var[:], msq[:], m2[:], AOT.subtract)
            vare = ppool.tile([C, 1], dt.float32)
            nc.vector.tensor_scalar(vare[:], var[:], EPS, None, AOT.add)
            sd = ppool.tile([C, 1], dt.float32)
            nc.scalar.activation(sd[:], vare[:], AFT.Sqrt)
            inv = ppool.tile([C, 1], dt.float32)
            nc.vector.reciprocal(inv[:], sd[:])
            scl = ppool.tile([C, 1], dt.float32)
            nc.vector.tensor_tensor(scl[:], bnc_s[:, 0:1], inv[:], AOT.mult)
            mt = ppool.tile([C, 1], dt.float32)
            nc.vector.tensor_tensor(mt[:], mean[:], scl[:], AOT.mult)
            bia = ppool.tile([C, 1], dt.float32)
            nc.vector.tensor_tensor(bia[:], bnc_s[:, 1:2], mt[:], AOT.subtract)

            ov = out_d.rearrange("c (n q) -> c n q", q=128)
            for (b0, nb, slot) in SEGS:
                npix = 128 * nb
                on = wpool.tile([C, NPIX], dt.float32, tag="on", bufs=2)
                nc.scalar.activation(on[:, 0:npix],
                                     conv_s[:, 128 * b0: 128 * b0 + npix],
                                     AFT.Relu, bias=bia[:], scale=scl[:])
                onv = on[:, 0:npix].rearrange("c (n q) -> c n q", q=128)
                # even local blocks -> even global block slots, odd -> odd
                nc.sync.dma_start(ov[:, b0: b0 + nb: 2, :],
                                  onv[:, 0:nb // 2, :])
                nc.sync.dma_start(ov[:, b0 + 1: b0 + nb: 2, :],
                                  onv[:, nb // 2:nb, :])


def _prep_core(xb, w_off, b_off, w_def, gamma, beta):
    """Host-side input prep for one batch item. xb: [64, 96, 96] f32."""
    bf16 = ml_dtypes.bfloat16
    ins = {}
    # xpad: zero-pad by 1 for the 3x3 offset conv (bf16)
    xp = np.zeros((C, 98, 98), np.float32)
    xp[:, 1:97, 1:97] = xb
    ins["xpad"] = xp.reshape(C, 98 * 98).astype(bf16)
    # x2pad: 4-corner gather table, channel-interleaved bf16.
    # row q = y0*PW + x0 holds [xz[y0,x0,c], xz[y0+1,x0,c], xz[y0,x0+1,c],
    # xz[y0+1,x0+1,c]] for c in 0..63 -> 256 bf16 = 512B
    xz = np.zeros((PR + 2, PW, C), np.float32)
    xz[PADM:PADM + H, PADM:PADM + W] = xb.transpose(1, 2, 0)
    xzf = xz.reshape((PR + 2) * PW, C)
    tab = np.stack([xzf[0:NQ], xzf[PW:NQ + PW], xzf[1:NQ + 1],
                    xzf[PW + 1:NQ + PW + 1]], axis=2)  # [NQ, C, 4]
    ins["x2pad"] = tab.reshape(NQ, 4 * C).astype(bf16)
    # weight rearrangements
    wofft = np.zeros((C, K2 * 18), np.float32)
    for k in range(K2):
        wofft[:, 18 * k:18 * k + 18] = w_off[:, :, k // K, k % K].T
    ins["woffT"] = wofft.astype(bf16)
    wdeft = np.zeros((128, K2 * C), np.float32)
    for k in range(K2):
        blk = w_def[:, :, k // K, k % K].T  # [cin, cout]
        wdeft[0:64, C * k:C * k + C] = blk
        wdeft[64:128, C * k:C * k + C] = blk
    ins["wdefT"] = wdeft.astype(bf16)
    # base grids (pixel-major [128, 72] per tap), fold b_off and pad margin
    pixi = np.arange(HW, dtype=np.int64)
    ygrid = (pixi // W).astype(np.float32)
    xgrid = (pixi % W).astype(np.float32)
    ypm = ygrid.reshape(NB, 128).T    # [p, b] pixel-major
    xpm = xgrid.reshape(NB, 128).T
    bpy = np.zeros((128, K2 * NB), np.float32)
    bpx = np.zeros((128, K2 * NB), np.float32)
    for k in range(K2):
        ky, kx = k // K - 1, k % K - 1
        bpy[:, NB * k:NB * k + NB] = ypm + (ky + PADM + b_off[2 * k])
        bpx[:, NB * k:NB * k + NB] = xpm + (kx + PADM + b_off[2 * k + 1])
    ins["bpy"] = bpy
    ins["bpx"] = bpx
    ins["ident"] = np.eye(128, dtype=np.float32)
    ins["bnc"] = np.stack([gamma, beta], axis=1).astype(np.float32)
    return ins


def _get_nc():
    if "nc" not in _CACHE:
        nc = bacc.Bacc("TRN2", target_bir_lowering=False, debug=False,
                       num_devices=NCORES, num_swdge_queues=2)
        _build(nc)
        nc.compile()
        _CACHE["nc"] = nc
    return _CACHE["nc"]


def kernel(x, w_off, b_off, w_def, b_def, gamma, beta, trace=False, tmpdir=None):
    x = np.asarray(x, np.float32)
    w_off = np.asarray(w_off, np.float32)
    b_off = np.asarray(b_off, np.float32)
    w_def = np.asarray(w_def, np.float32)
    gamma = np.asarray(gamma, np.float32)
    beta = np.asarray(beta, np.float32)
    # b_def cancels exactly in training-mode BN; accepted but unused.
    nc = _get_nc()
    in_maps = [_prep_core(x[b], w_off, b_off, w_def, gamma, beta)
               for b in range(B)]
    res = bass_utils.run_bass_kernel_spmd(
        nc, in_maps, core_ids=list(range(NCORES)), trace=trace, tmpdir=tmpdir)
    out = np.stack([res.results[b]["out"].reshape(C, H, W) for b in range(B)])
    if trace:
        kernel.last_exec_time_ns = res.exec_time_ns
        kernel.last_results = res
    return out
